# revision 18
# baseline (speedup 1.0000x reference)
"""Dropless MoE FFN (router + top-2 dispatch + per-expert MLP + combine) on
8 Trainium2 NeuronCores.

Strategy (expert parallelism, per the sharding hint):
  - Router (softmax + top-2) runs on host in fp32 — it is ~0.02% of the
    FLOPs and IS the token dispatch: each of the 8 cores owns one expert
    and receives only the tokens routed to it (gather on host replaces the
    device all-to-all; full inputs in / full output out per the contract).
  - Mixed precision split (key speedup): each expert's routed tokens are
    ranked by their router combine weight.  The Cb highest-weight tokens
    run the 2-layer GELU MLP in bf16; the overflow (n_e - Cb tokens, the
    LOWEST combine weights, whose output error is downweighted in the
    final combine) runs in pure fp8e4 using DoubleRow double-pumped
    matmuls (K=256 per instruction => 2x bf16 MAC rate, measured on HW).
    Block sizes (Cb, Cf) are fixed across cores so one SPMD program
    serves all 8; padding lands in the cheap fp8 block.  Emulated
    end-to-end rel err 1.6e-2 vs the 2e-2 budget (bf16-only is 3.4e-3).
  - Host applies the combine weights (with the fp8 dequant folded in) and
    scatter-adds the two expert outputs per token.

Device kernel layout per core:
  Both GEMMs keep tokens on the MOVING (free) dim — GEMM1 produces
  hT[f_tile, tokens], GEMM2 produces yT[d_tile, tokens] — so GELU is
  applied PSUM->SBUF on ScalarE with no transposes.  fp8 operands are
  packed host-side in DoubleRow pair layout ([128, 2, N]: two K-chunks
  interleaved per instruction).  The fp8 GEMM1 psum carries scale
  16*256 = 4096 (folded into the GELU input scale); h is stored in e4m3
  directly (s_h=1) and the GEMM2 fp8 psum carries scale 256 (folded into
  the host-side combine weights).  Head DMAs are split across both HWDGE
  rings (sync + scalar) and interleaved (w1 chunk, xt chunk) so the PE
  starts within ~2us; w1-fp8 rides sync paced behind GEMM1 progress, and
  w2/w2-fp8 prefetch rides the SWDGE ring, dependency-paced so the 12.6MB
  of GEMM2 weights never steal HBM bandwidth from the critical head.
"""

import sys

for _p in ("/opt/trn_rl_repo",):
    if _p not in sys.path:
        sys.path.insert(0, _p)

import numpy as np
import ml_dtypes

BF16 = ml_dtypes.bfloat16
E4 = ml_dtypes.float8_e4m3   # IEEE e4m3 (max 240) == TRN FP8_EXP4

D_MODEL = 1024
D_FFN = 4096
N_EXPERTS = 8
TOP_K = 2
N_CORES = 8
P = 128                 # SBUF/PSUM partitions
KC = D_MODEL // P       # 8 contraction chunks for GEMM1
FC = D_FFN // P         # 32 f-chunks (contraction chunks for GEMM2)

_kernel_cache: dict[tuple, object] = {}


def _token_groups(C):
    """Split C token columns into <=512-wide PSUM-bank-sized groups."""
    n_g = -(-C // 512)
    base, rem = divmod(C, n_g)
    sizes = [base + (1 if g < rem else 0) for g in range(n_g)]
    groups = []
    off = 0
    for sz in sizes:
        groups.append((off, sz))
        off += sz
    return groups


def _build(Cb, Cf):
    import concourse.bass as bass
    import concourse.mybir as mybir
    import concourse.tile as tile
    from concourse.tile_rust import add_dep_helper
    from concourse import bacc

    dt = mybir.dt
    AF = mybir.ActivationFunctionType
    DR = mybir.MatmulPerfMode.DoubleRow
    C = Cb + Cf
    groups = _token_groups(Cb)

    nc = bacc.Bacc("TRN2", target_bir_lowering=False, debug=False,
                   num_devices=N_CORES)
    xt_d = nc.dram_tensor("xt", [KC, P, Cb], dt.bfloat16,
                          kind="ExternalInput").ap()
    x8_d = nc.dram_tensor("x8", [KC // 2, P, 2 * Cf], dt.float8e4,
                          kind="ExternalInput").ap()
    w1_d = nc.dram_tensor("w1", [KC, P, D_FFN], dt.bfloat16,
                          kind="ExternalInput").ap()
    w18_d = nc.dram_tensor("w18", [FC, P, KC * P], dt.float8e4,
                           kind="ExternalInput").ap()
    w2_d = nc.dram_tensor("w2", [FC // 4, P, 4, D_MODEL], dt.bfloat16,
                          kind="ExternalInput").ap()
    w28_d = nc.dram_tensor("w28", [FC // 2, P, 2 * D_MODEL], dt.float8e4,
                           kind="ExternalInput").ap()
    y_d = nc.dram_tensor("y", [KC, P, C], dt.float32,
                         kind="ExternalOutput").ap()

    with tile.TileContext(nc) as tc:
        with (
            tc.tile_pool(name="xt", bufs=KC) as xt_pool,
            tc.tile_pool(name="x8", bufs=KC // 2) as x8_pool,
            tc.tile_pool(name="w1", bufs=2 * KC) as w1_pool,
            tc.tile_pool(name="w18", bufs=8) as w18_pool,
            tc.tile_pool(name="w2", bufs=FC // 4) as w2_pool,
            tc.tile_pool(name="w28", bufs=FC // 2) as w28_pool,
            tc.tile_pool(name="ht", bufs=FC // 4) as ht_pool,
            tc.tile_pool(name="h8", bufs=FC // 2) as h8_pool,
            tc.tile_pool(name="yo", bufs=1) as y_pool,
            tc.tile_pool(name="ps", bufs=8, space=bass.MemorySpace.PSUM) as ps_pool,
        ):
            # w1 streamed in m-blocks (narrow first blocks shrink the
            # critical head bytes so the PE never starves during m=0/1);
            # later blocks paced naturally by slot-WAR.
            BLOCKS = [(0, 2), (2, 2), (4, 4), (8, 4), (12, 4), (16, 4),
                      (20, 4), (24, 4), (28, 4)]
            blk_of_m = {}
            for b, (m0, bm) in enumerate(BLOCKS):
                for m in range(m0, m0 + bm):
                    blk_of_m[m] = (b, m0, bm)
            # Head: interleave (w1 block0 chunk -> sync ring, xt chunk ->
            # scalar ring) so GEMM1 m=0 can start as chunks land; the fp8
            # moving tiles ride between the xt chunks so m=0's DoubleRow
            # chain is never the straggler.
            w1_first = []
            w1_b1 = []
            xt_t = []
            for kc in range(KC):
                ring_w = nc.sync if kc % 2 == 0 else nc.scalar
                ring_x = nc.scalar if kc % 2 == 0 else nc.sync
                w = w1_pool.tile([P, 2 * P], dt.bfloat16, tag="w1",
                                 name=f"w1_0_{kc}")
                ring_w.dma_start(w[:], w1_d[kc][:, :2 * P])
                w1_first.append(w)
                t = xt_pool.tile([P, Cb], dt.bfloat16, tag="xt",
                                 name=f"xt_{kc}")
                ring_x.dma_start(t[:], xt_d[kc])
                xt_t.append(t)
                if kc == 3:
                    x8_t = []
                    for j in range(KC // 2):
                        t8 = x8_pool.tile([P, 2 * Cf], dt.float8e4, tag="x8",
                                          name=f"x8_{j}")
                        nc.gpsimd.dma_start(t8[:], x8_d[j])
                        x8_t.append(t8)
            for kc in range(KC):
                ring_w = nc.sync if kc % 2 == 0 else nc.scalar
                w = w1_pool.tile([P, 2 * P], dt.bfloat16, tag="w1",
                                 name=f"w1_1_{kc}")
                ring_w.dma_start(w[:], w1_d[kc][:, 2 * P:4 * P])
                w1_b1.append(w)
            w18_t = {}
            for m in range(3):
                t = w18_pool.tile([P, KC * P], dt.float8e4, tag="w18",
                                  name=f"w18_{m}")
                nc.gpsimd.dma_start(t[:], w18_d[m])
                w18_t[m] = t

            # ---- GEMM1: hT[m*128+p, t] = sum_k w1[k, f] * x[t, k], + GELU
            ht_t = []
            h8_t = []
            gelu_insts = []
            w1_t = w1_first
            for m in range(FC):
                b, m0, bm = blk_of_m[m]
                mi = m - m0
                if m + 2 in blk_of_m:
                    b2, m02, bm2 = blk_of_m[m + 2]
                    if m + 2 == m02 and b2 > 1:
                        w1_next = [w1_pool.tile([P, bm2 * P], dt.bfloat16,
                                                tag="w1", name=f"w1_{b2}_{kc}")
                                   for kc in range(KC)]
                        for kc in range(KC):
                            ring = nc.sync if kc % 2 == 0 else nc.scalar
                            ring.dma_start(
                                w1_next[kc][:],
                                w1_d[kc][:, m02 * P:(m02 + bm2) * P])
                        w1_pending = w1_next
                if mi == 0 and b == 1:
                    w1_t = w1_b1
                elif mi == 0 and b > 1:
                    w1_t = w1_pending
                if m + 3 < FC:
                    t = w18_pool.tile([P, KC * P], dt.float8e4,
                                      tag="w18", name=f"w18_{m + 3}")
                    dma = nc.scalar.dma_start(t[:], w18_d[m + 3])
                    if gelu_insts:
                        add_dep_helper(dma.ins, gelu_insts[-1].ins, sync=True,
                                       reason="pace w18 behind GEMM1 progress")
                    w18_t[m + 3] = t

                ps = [ps_pool.tile([P, 512], dt.float32, tag="ps1",
                                   name=f"ps1_{m}_{g}")
                      for g in range(len(groups))]
                ps8 = ps_pool.tile([P, 512], dt.float32, tag="ps1",
                                   name=f"ps18_{m}")
                for j in range(KC // 2):
                    nc.tensor.matmul(ps8[:, :Cf],
                                     w18_t[m][:, j * 2 * P:(j + 1) * 2 * P]
                                     .rearrange('p (s c) -> p s c', s=2, c=P),
                                     x8_t[j][:].rearrange('p (s c) -> p s c',
                                                          s=2, c=Cf),
                                     start=(j == 0), stop=(j == KC // 2 - 1),
                                     perf_mode=DR)
                for kc in range(KC):
                    lhsT = w1_t[kc][:, mi * P:(mi + 1) * P]
                    for g, (off, sz) in enumerate(groups):
                        nc.tensor.matmul(ps[g][:, :sz], lhsT,
                                         xt_t[kc][:, off:off + sz],
                                         start=(kc == 0), stop=(kc == KC - 1))
                if m % 4 == 0:
                    ht = ht_pool.tile([P, 4, Cb], dt.bfloat16, tag="ht",
                                      name=f"ht_{m // 4}")
                    ht_t.append(ht)
                if m % 2 == 0:
                    h8 = h8_pool.tile([P, 2, Cf], dt.float8e4, tag="h8",
                                      name=f"h8_{m // 2}")
                    h8_t.append(h8)
                gelu_inst = None
                for g, (off, sz) in enumerate(groups):
                    gelu_inst = nc.scalar.activation(ht[:, m % 4, off:off + sz],
                                                     ps[g][:, :sz],
                                                     AF.Gelu_apprx_tanh)
                gelu_insts.append(gelu_inst)
                nc.scalar.activation(h8[:, m % 2, :], ps8[:, :Cf],
                                     AF.Gelu_apprx_tanh, scale=1.0 / 4096.0)

            # w2 (+ fp8 w2) prefetch on the SWDGE ring, each chunk paced
            # behind a later GELU so the 12.6MB never steals HBM bandwidth
            # from the critical head loads; all chunks land before GEMM2
            # needs them
            w2_t = []
            for j in range(FC // 4):
                w2t = w2_pool.tile([P, 4, D_MODEL], dt.bfloat16, tag="w2",
                                   name=f"w2_{j}")
                w2_dma = nc.gpsimd.dma_start(w2t[:], w2_d[j])
                pace = 6 + (j * 23) // max(FC // 4 - 1, 1)
                add_dep_helper(w2_dma.ins, gelu_insts[pace].ins, sync=True,
                               reason="pace w2 prefetch behind GEMM1 progress")
                w2_t.append(w2t)
            w28_t = []
            for j in range(FC // 2):
                w28t = w28_pool.tile([P, 2 * D_MODEL], dt.float8e4, tag="w28",
                                     name=f"w28_{j}")
                w28_dma = nc.gpsimd.dma_start(w28t[:], w28_d[j])
                pace = 24 + (j * 7) // max(FC // 2 - 1, 1)
                add_dep_helper(w28_dma.ins, gelu_insts[pace].ins, sync=True,
                               reason="pace w28 prefetch behind GEMM1 tail")
                w28_t.append(w28t)

            # ---- GEMM2 (flipped): yT[dc*128+p, t] = sum_f w2[f, d] * h[t, f]
            # Tokens ride the moving dim.  Within each weight position the
            # smallest group goes FIRST so the next position's LDWEIGHTS
            # always hides under a long matmul; the fp8 DoubleRow chain runs
            # last.  Combine weights are applied on the host.
            # Per dc: the fp8 DoubleRow chain runs FIRST so its PSUM->SBUF
            # copy + DMA overlap the long bf16 fc loop; bf16 groups run
            # big-first so the final position ends on the small group, whose
            # copy + DMA are the only exposed tail.  Output is copied and
            # DMA'd per piece to keep the post-matmul tail minimal.
            g_order = sorted(range(len(groups)), key=lambda g: -groups[g][1])
            for dc in range(KC):
                psg = [ps_pool.tile([P, 512], dt.float32, tag="ps1",
                                    name=f"psy_{dc}_{g}")
                       for g in range(len(groups))]
                ps8 = ps_pool.tile([P, 512], dt.float32, tag="ps1",
                                   name=f"psy8_{dc}")
                y_t = y_pool.tile([P, C], dt.float32, tag="yo")
                for fc in range(FC):
                    lhsT = w2_t[fc // 4][:, fc % 4, dc * P:(dc + 1) * P]
                    for g in g_order:
                        off, sz = groups[g]
                        nc.tensor.matmul(psg[g][:, :sz], lhsT,
                                         ht_t[fc // 4][:, fc % 4, off:off + sz],
                                         start=(fc == 0), stop=(fc == FC - 1))
                # fp8 chain last: the bf16 copies + piece DMAs overlap it, so
                # only the small f8 copy + its 132KB DMA trail the final MM
                for j in range(FC // 2):
                    nc.tensor.matmul(ps8[:, :Cf],
                                     w28_t[j][:].rearrange(
                                         'p (s d) -> p s d', s=2, d=D_MODEL)
                                     [:, :, dc * P:(dc + 1) * P],
                                     h8_t[j][:],
                                     start=(j == 0), stop=(j == FC // 2 - 1),
                                     perf_mode=DR)
                for g in g_order:
                    off, sz = groups[g]
                    nc.scalar.activation(y_t[:, off:off + sz], psg[g][:, :sz],
                                         AF.Copy)
                    nc.sync.dma_start(y_d[dc][:, off:off + sz],
                                      y_t[:, off:off + sz])
                nc.scalar.activation(y_t[:, Cb:C], ps8[:, :Cf], AF.Copy)
                nc.sync.dma_start(y_d[dc][:, Cb:C], y_t[:, Cb:C])

    nc.compile()
    return nc


def _route(x, router_w):
    """Replicate the reference router math (jax on CPU, fp32)."""
    import jax
    import jax.numpy as jnp

    with jax.default_device(jax.devices("cpu")[0]):
        xt = jnp.asarray(np.asarray(x, np.float32)).reshape(-1, D_MODEL)
        logits = xt @ jnp.asarray(np.asarray(router_w, np.float32))
        probs = jax.nn.softmax(logits, axis=-1)
        top_p, top_i = jax.lax.top_k(probs, TOP_K)
    return np.asarray(top_p), np.asarray(top_i)


def _run(x, router_w, w1, w2, trace=False):
    from concourse import bass_utils

    x = np.asarray(x, np.float32)
    w1 = np.asarray(w1, np.float32)
    w2 = np.asarray(w2, np.float32)
    B, S, _ = x.shape
    T = B * S
    xt = x.reshape(T, D_MODEL)

    top_p, top_i = _route(x, router_w)

    idxs, wts = [], []
    maxn = 0
    for e in range(N_EXPERTS):
        hit = top_i == e                       # [T, K]
        sel = hit.any(axis=1)
        idx = np.nonzero(sel)[0]
        w = (top_p * hit).sum(axis=1)[sel]     # combine weight per routed token
        idxs.append(idx)
        wts.append(w.astype(np.float32))
        maxn = max(maxn, len(idx))

    C = max(maxn, 4 * P)
    # fixed block split: Cf lowest-combine-weight columns per expert in fp8
    Cf = min(512, max(P, C - 832))
    Cb = C - Cf
    nc = _kernel_cache.get((Cb, Cf))
    if nc is None:
        nc = _build(Cb, Cf)
        _kernel_cache[(Cb, Cf)] = nc

    in_maps = []
    splits = []
    for e in range(N_EXPERTS):
        idx, w = idxs[e], wts[e]
        n = len(idx)
        nf8 = max(0, n - Cb)
        order = np.argsort(w)                  # ascending combine weight
        f8_i, bf_i = order[:nf8], order[nf8:]
        splits.append((bf_i, f8_i))

        xb = np.zeros((Cb, D_MODEL), np.float32)
        xb[:len(bf_i)] = xt[idx[bf_i]]
        xtb = np.ascontiguousarray(xb.T).astype(BF16).reshape(KC, P, Cb)

        x8r = np.zeros((Cf, D_MODEL), np.float32)
        x8r[:nf8] = xt[idx[f8_i]]
        x8q = np.asarray(x8r.T * 16.0, E4)           # [D, Cf]
        x8b = np.ascontiguousarray(
            x8q.reshape(KC // 2, 2, P, Cf).transpose(0, 2, 1, 3))

        w1b = np.ascontiguousarray(w1[e].astype(BF16).reshape(KC, P, D_FFN))
        w18q = np.asarray(w1[e] * 256.0, E4)          # [D, F]
        w18b = np.ascontiguousarray(
            w18q.reshape(KC // 2, 2, P, FC, P).transpose(3, 2, 0, 1, 4))

        w2b = np.ascontiguousarray(w2[e].astype(BF16)
                                   .reshape(FC // 4, 4, P, D_MODEL)
                                   .transpose(0, 2, 1, 3))
        w28q = np.asarray(w2[e] * 256.0, E4)          # [F, D]
        w28b = np.ascontiguousarray(
            w28q.reshape(FC // 2, 2, P, D_MODEL).transpose(0, 2, 1, 3))

        in_maps.append({"xt": xtb, "x8": x8b, "w1": w1b, "w18": w18b,
                        "w2": w2b, "w28": w28b})

    res = bass_utils.run_bass_kernel_spmd(
        nc, in_maps, core_ids=list(range(N_CORES)), trace=trace)

    out = np.zeros((T, D_MODEL), np.float32)
    for e in range(N_EXPERTS):
        idx, w = idxs[e], wts[e]
        bf_i, f8_i = splits[e]
        yt = np.asarray(res.results[e]["y"], np.float32).reshape(D_MODEL,
                                                                 Cb + Cf).T
        if len(bf_i):
            out[idx[bf_i]] += w[bf_i][:, None] * yt[:len(bf_i)]
        if len(f8_i):
            out[idx[f8_i]] += (w[f8_i] / 256.0)[:, None] * \
                yt[Cb:Cb + len(f8_i)]
    return out.reshape(B, S, D_MODEL), res


def kernel(**inputs):
    out, _ = _run(inputs["x"], inputs["router_w"], inputs["w1"], inputs["w2"])
    return out
